# revision 1
# baseline (speedup 1.0000x reference)
"""NeighborMLPConvLayer Trainium2 kernel.

Strategy (8 NeuronCores, SPMD, edge-parallel):
  - Edges are split into 8 equal contiguous ranges (edges are sorted by
    destination segment, so each core covers a contiguous span of output
    rows; boundary segments are fixed up by a host-side overlap-add).
  - Per core, edges are packed into fixed-capacity "windows" of 2048 slots
    (1024 for neighbor-index < SPLIT, 1024 for >= SPLIT, padded with a
    zero-row index and weight 0).  A window never spans more than 128
    distinct segments, so its segment-sum accumulates into one PSUM tile.
  - Features are fetched with transpose-mode dma_gather from bf16 tables
    padded to 128 columns (256B rows), landing feature-major [ch, edge].
  - MLP: h = gelu(W1a.T@rep_T + W1b.T@slf_T + b1) accumulated in PSUM,
    y = h'.T @ W2 via per-128-column stationary-operand matmuls (pivots
    edges onto partitions), y scaled by 1/count, then segment-summed via a
    one-hot matmul built on-chip (iota == seg_local).
  - Window results land in per-window output slots; the host overlap-adds
    slots into the final [M, 64] output and applies the b2 bias.
"""

import sys

sys.path.insert(0, "/opt/trn_rl_repo")

import numpy as np
import ml_dtypes

BF16 = ml_dtypes.bfloat16
FP8 = ml_dtypes.float8_e4m3

# Problem geometry (hardcoded per the task contract).
N = 50000
M = 50000
C = 32
H = 128
O = 64
E = 1_600_000
NCORES = 8

SPLIT = 25000          # lo/hi table split (int16 gather index limit)
WIN = 2048             # slots per window
HALF = 1024            # lo-slot budget (== hi budget)
TILE = 512             # edge-slots per M1 tile
CH = 128               # edge-slots per chunk (partition dim)
GRP = 2                # windows per gather group
SINGLE_PACKET = False  # single-packet mode breaks >~1k descriptors
ABLATE = set()          # timing-attribution knobs (break correctness)

_prog_cache = {}


# ----------------------------------------------------------------- host prep

def _wrap_idx(a):
    """[n] int16 -> [128, n//16] gather index layout (16-wrap, 8x replica)."""
    t = a.reshape(-1, 16).T
    return np.ascontiguousarray(np.tile(t, (8, 1)))


def _part_major(a, dt):
    """[n] -> [128, n//128]; slot j*128+p -> [p, j]."""
    return np.ascontiguousarray(a.reshape(-1, 128).T.astype(dt))


def _build_windows(idx_c, seg_c, w_c, nwin):
    """Pack one core's edges into fixed windows.

    Returns per-core blobs: gather indices (lo/hi/slf), seg_local (bf16),
    w (f32), and flush metadata (base segment + span per window).
    """
    nloc = idx_c.shape[0]
    islo = idx_c < SPLIT
    cum_lo = np.zeros(nloc + 1, np.int64)
    np.cumsum(islo, out=cum_lo[1:])
    cum_hi = np.zeros(nloc + 1, np.int64)
    np.cumsum(~islo, out=cum_hi[1:])

    seg_base = int(seg_c[0])

    gl = np.full(nwin * HALF, SPLIT, np.int16)        # Z row of tab_lo
    gh = np.full(nwin * HALF, N - SPLIT, np.int16)    # Z row of tab_hi
    gs_z = None                                       # filled later (s_tab-1)
    gs = np.zeros(nwin * WIN, np.int64)
    gs_pad = np.zeros(nwin * WIN, bool)
    segloc = np.zeros(nwin * WIN, np.float32)
    warr = np.zeros(nwin * WIN, np.float32)
    bases = np.zeros(nwin, np.int64)
    spans = np.zeros(nwin, np.int64)

    pos = 0
    wi = 0
    while pos < nloc:
        assert wi < nwin, "window budget exceeded"
        b0 = int(seg_c[pos])
        p_span = int(np.searchsorted(seg_c, b0 + 128, side="left"))
        p_lo = int(np.searchsorted(cum_lo, cum_lo[pos] + HALF, side="right")) - 1
        p_hi = int(np.searchsorted(cum_hi, cum_hi[pos] + HALF, side="right")) - 1
        cut = min(p_span, p_lo, p_hi, nloc)
        assert cut > pos
        sel = slice(pos, cut)
        m = islo[sel]
        lo_i = idx_c[sel][m]
        hi_i = idx_c[sel][~m] - SPLIT
        o = wi * HALF
        gl[o : o + lo_i.shape[0]] = lo_i.astype(np.int16)
        gh[o : o + hi_i.shape[0]] = hi_i.astype(np.int16)
        s_lo = seg_c[sel][m]
        s_hi = seg_c[sel][~m]
        o2 = wi * WIN
        nl, nh = s_lo.shape[0], s_hi.shape[0]
        segloc[o2 : o2 + nl] = s_lo - b0
        segloc[o2 + HALF : o2 + HALF + nh] = s_hi - b0
        warr[o2 : o2 + nl] = w_c[sel][m]
        warr[o2 + HALF : o2 + HALF + nh] = w_c[sel][~m]
        gs[o2 : o2 + nl] = s_lo - seg_base
        gs[o2 + HALF : o2 + HALF + nh] = s_hi - seg_base
        gs_pad[o2 + nl : o2 + HALF] = True
        gs_pad[o2 + HALF + nh : o2 + 2 * HALF] = True
        bases[wi] = b0
        spans[wi] = int(seg_c[cut - 1]) - b0 + 1
        pos = cut
        wi += 1

    # fully padded trailing windows
    gs_pad[wi * WIN :] = True

    span_tab = int(seg_c[-1]) - seg_base + 1
    return dict(
        gl=gl, gh=gh, gs=gs, gs_pad=gs_pad,
        segloc=segloc, warr=warr,
        bases=bases, spans=spans, n_real=wi,
        seg_base=seg_base, span_tab=span_tab,
    )


def _host_prep(in_features, out_features, W1, b1, W2, b2,
               neighbors_index, neighbors_row_splits):
    rs = np.asarray(neighbors_row_splits).astype(np.int64)
    idx_all = np.asarray(neighbors_index).astype(np.int64)
    counts = np.diff(rs)
    seg_ids = np.repeat(np.arange(M, dtype=np.int64), counts)
    w_seg = (1.0 / np.maximum(counts, 1)).astype(np.float32)
    w_edge = w_seg[seg_ids]

    bounds = [round(k * E / NCORES) for k in range(NCORES + 1)]

    # First pass: window counts per core so the program shape is uniform.
    cores = []
    for k in range(NCORES):
        lo, hi = bounds[k], bounds[k + 1]
        cores.append((idx_all[lo:hi], seg_ids[lo:hi], w_edge[lo:hi]))

    # conservative shared window count
    nwin_est = 0
    built = []
    for idx_c, seg_c, w_c in cores:
        b = _build_windows(idx_c, seg_c, w_c, nwin=(idx_c.shape[0] // HALF + 4))
        built.append(b)
        nwin_est = max(nwin_est, b["n_real"])
    nwin = -(-nwin_est // GRP) * GRP

    s_tab = max(b["span_tab"] for b in built) + 1  # +1 zero row
    assert s_tab < 32768

    # Tables (bf16, rows padded to 128 cols; last row zeros).
    tab_lo = np.zeros((SPLIT + 1, 128), BF16)
    tab_lo[:SPLIT, :C] = in_features[:SPLIT]
    tab_hi = np.zeros((N - SPLIT + 1, 128), BF16)
    tab_hi[: N - SPLIT, :C] = in_features[SPLIT:]

    w1 = np.asarray(W1, np.float32)
    w1b1 = np.concatenate([w1[C:], np.asarray(b1, np.float32).reshape(1, H)], 0)
    consts = dict(
        w1a=np.ascontiguousarray(w1[:C]).astype(BF16),
        w1b1=np.ascontiguousarray(w1b1).astype(BF16),
        w2=np.asarray(W2, np.float32).astype(BF16),
    )

    in_maps = []
    metas = []
    for k in range(NCORES):
        b = built[k]
        nw = nwin
        # per-window outF.T blocks [33, nw*128]: cols = segs b0..b0+128,
        # row C (=32) is ones so W1b' row C injects b1 into q.
        outft = np.zeros((C + 1, nw * 128), BF16)
        outf32 = np.asarray(out_features, np.float32)
        for wi in range(b["n_real"]):
            base = int(b["bases"][wi])
            span = min(128, M - base)
            blk = outf32[base : base + span].T.astype(BF16)
            outft[:C, wi * 128 : wi * 128 + span] = blk
            outft[C, wi * 128 : (wi + 1) * 128] = 1.0
        # one-hot S.T [128, nw*WIN] fp8: st[s, j] = (seg_local[j] == s)
        sl_all = np.zeros(nw * WIN, np.int32)
        sl_all[: b["segloc"].shape[0]] = b["segloc"][: nw * WIN].astype(np.int32)
        st_valid = np.zeros(nw * WIN, bool)
        nreal_slots = min(b["warr"].shape[0], nw * WIN)
        st_valid[:nreal_slots] = b["warr"][:nreal_slots] > 0
        st = (np.arange(128, dtype=np.int32)[:, None] == sl_all[None, :]) & st_valid[None, :]
        st = st.astype(FP8)
        # edge-major one-hot S [128 e, chunk-major 128 s] for M3 lhsT
        nchunks = nw * WIN // 128
        sl3 = sl_all.reshape(nchunks, 128).T            # [128 e, chunk]
        v3 = st_valid.reshape(nchunks, 128).T
        sme = (sl3[:, :, None] == np.arange(128, dtype=np.int32)[None, None, :]) & v3[:, :, None]
        sme = np.ascontiguousarray(sme.reshape(128, nchunks * 128)).astype(FP8)
        gl = np.full(nw * HALF, SPLIT, np.int16)
        gl[: b["gl"].shape[0]] = b["gl"][: nw * HALF]
        gh = np.full(nw * HALF, N - SPLIT, np.int16)
        gh[: b["gh"].shape[0]] = b["gh"][: nw * HALF]
        sl = np.zeros(nw * WIN, np.float32)
        sl[: b["segloc"].shape[0]] = b["segloc"][: nw * WIN]
        wa = np.zeros(nw * WIN, np.float32)
        wa[: b["warr"].shape[0]] = b["warr"][: nw * WIN]

        in_maps.append(dict(
            tab_lo=tab_lo,
            tab_hi=tab_hi,
            outft=outft,
            st=st,
            sme=sme,
            idx_lo=_wrap_idx(gl),
            idx_hi=_wrap_idx(gh),
            w_arr=_part_major(wa, np.float32),
            **consts,
        ))
        metas.append(b)

    return in_maps, metas, nwin, s_tab, counts


# ------------------------------------------------------------ device program

def _build_program(nwin, s_tab):
    import concourse.bacc as bacc
    import concourse.bass as bass
    import concourse.mybir as mybir
    import concourse.tile as tile

    dt = mybir.dt
    nc = bacc.Bacc("TRN2", target_bir_lowering=False, debug=False)

    d_tab_lo = nc.dram_tensor("tab_lo", [SPLIT + 1, 128], dt.bfloat16,
                              kind="ExternalInput")
    d_tab_hi = nc.dram_tensor("tab_hi", [N - SPLIT + 1, 128], dt.bfloat16,
                              kind="ExternalInput")
    d_outft = nc.dram_tensor("outft", [C + 1, nwin * 128], dt.bfloat16,
                             kind="ExternalInput")
    d_st = nc.dram_tensor("st", [128, nwin * WIN], dt.float8e4,
                          kind="ExternalInput")
    d_idx_lo = nc.dram_tensor("idx_lo", [128, nwin * HALF // 16], dt.int16,
                              kind="ExternalInput")
    d_idx_hi = nc.dram_tensor("idx_hi", [128, nwin * HALF // 16], dt.int16,
                              kind="ExternalInput")
    d_sme = nc.dram_tensor("sme", [128, nwin * WIN], dt.float8e4,
                           kind="ExternalInput")
    d_w = nc.dram_tensor("w_arr", [128, nwin * WIN // 128], dt.float32,
                         kind="ExternalInput")
    d_w1a = nc.dram_tensor("w1a", [C, H], dt.bfloat16, kind="ExternalInput")
    d_w1b1 = nc.dram_tensor("w1b1", [C + 1, H], dt.bfloat16, kind="ExternalInput")
    d_w2 = nc.dram_tensor("w2", [H, O], dt.bfloat16, kind="ExternalInput")
    d_out = nc.dram_tensor("out_slots", [nwin * 128, O], dt.float32,
                           kind="ExternalOutput")

    n_tiles = WIN // TILE            # tiles per window
    n_ch = TILE // CH                # chunks per tile
    lo_tiles = HALF // TILE          # leading tiles sourced from the lo gather

    from contextlib import ExitStack

    with tile.TileContext(nc) as tc, ExitStack() as ctx:
        cpool = ctx.enter_context(tc.tile_pool(name="consts", bufs=1))
        gpool = ctx.enter_context(tc.tile_pool(name="gather", bufs=3))
        wpool = ctx.enter_context(tc.tile_pool(name="work", bufs=3))
        fpool = ctx.enter_context(tc.tile_pool(name="flush", bufs=3))
        hpsum = ctx.enter_context(tc.tile_pool(name="hpsum", bufs=2, space="PSUM"))
        ypsum = ctx.enter_context(tc.tile_pool(name="ypsum", bufs=2, space="PSUM"))
        wpsum = ctx.enter_context(tc.tile_pool(name="wpsum", bufs=2, space="PSUM"))
        qpsum = ctx.enter_context(tc.tile_pool(name="qpsum", bufs=2, space="PSUM"))

        # ---- constants / resident data
        w1a_sb = cpool.tile([C, H], dt.bfloat16, tag="w1a")
        w1b1_sb = cpool.tile([C + 1, H], dt.bfloat16, tag="w1b1")
        w2_sb = cpool.tile([H, O], dt.bfloat16, tag="w2")
        outft_sb = cpool.tile([C + 1, nwin * 128], dt.bfloat16, tag="outft")
        ixlo_sb = cpool.tile([128, nwin * HALF // 16], dt.int16, tag="ixlo")
        ixhi_sb = cpool.tile([128, nwin * HALF // 16], dt.int16, tag="ixhi")
        w_sb = cpool.tile([128, nwin * WIN // 128], dt.float32, tag="w")

        nc.sync.dma_start(out=w1a_sb[:], in_=d_w1a[:])
        nc.sync.dma_start(out=w1b1_sb[:], in_=d_w1b1[:])
        nc.sync.dma_start(out=w2_sb[:], in_=d_w2[:])
        nc.sync.dma_start(out=outft_sb[:], in_=d_outft[:])
        nc.sync.dma_start(out=ixlo_sb[:], in_=d_idx_lo[:])
        nc.sync.dma_start(out=ixhi_sb[:], in_=d_idx_hi[:])
        nc.sync.dma_start(out=w_sb[:], in_=d_w[:])

        for g in range(nwin // GRP):
            glo = gpool.tile([128, 1, GRP * HALF], dt.bfloat16, tag="glo")
            ghi = gpool.tile([128, 1, GRP * HALF], dt.bfloat16, tag="ghi")
            st_sb = gpool.tile([128, GRP * WIN], dt.float8e4, tag="st")
            nc.scalar.dma_start(
                out=st_sb[:],
                in_=d_st[:, g * GRP * WIN : (g + 1) * GRP * WIN])
            sme_sb = gpool.tile([128, GRP * WIN], dt.float8e4, tag="sme")
            nc.scalar.dma_start(
                out=sme_sb[:],
                in_=d_sme[:, g * GRP * WIN : (g + 1) * GRP * WIN])
            c0 = g * GRP * HALF // 16
            c1 = (g + 1) * GRP * HALF // 16
            if "nogather" in ABLATE:
                for gt in (glo, ghi):
                    nc.gpsimd.dma_gather(
                        gt[:, :, 0:128], d_tab_lo[:], ixlo_sb[:, c0:c0 + 8],
                        num_idxs=128, num_idxs_reg=128,
                        elem_size=128, transpose=True,
                        single_packet=SINGLE_PACKET,
                    )
            else:
                nc.gpsimd.dma_gather(
                    glo[:], d_tab_lo[:], ixlo_sb[:, c0:c1],
                    num_idxs=GRP * HALF, num_idxs_reg=GRP * HALF,
                    elem_size=128, transpose=True, single_packet=SINGLE_PACKET,
                )
                nc.gpsimd.dma_gather(
                    ghi[:], d_tab_hi[:], ixhi_sb[:, c0:c1],
                    num_idxs=GRP * HALF, num_idxs_reg=GRP * HALF,
                    elem_size=128, transpose=True, single_packet=SINGLE_PACKET,
                )

            flst = fpool.tile([128, GRP, O], dt.float32, tag="flst")
            for wg in range(GRP):
                wi = g * GRP + wg
                win_ps = wpsum.tile([128, O], dt.float32, tag="win")
                # q = outF_win.T @ W1b + b1  (per window, [128 s, H])
                q_ps = qpsum.tile([128, H], dt.float32, tag="q")
                nc.tensor.matmul(
                    q_ps[:], lhsT=outft_sb[:, wi * 128 : (wi + 1) * 128],
                    rhs=w1b1_sb[:], start=True, stop=True,
                )
                q_sb = wpool.tile([128, H], dt.bfloat16, tag="q_sb")
                nc.vector.tensor_copy(out=q_sb[:], in_=q_ps[:])
                for t in range(n_tiles):
                    if "nomlp" in ABLATE:
                        continue
                    # ---- M1: h_pre = W1a.T @ rep_T + W1b.T @ slf_T
                    h_ps = hpsum.tile([128, TILE], dt.float32, tag="h")
                    if t < lo_tiles:
                        src = glo[0:C, 0,
                                  wg * HALF + t * TILE : wg * HALF + (t + 1) * TILE]
                    else:
                        tt = t - lo_tiles
                        src = ghi[0:C, 0,
                                  wg * HALF + tt * TILE : wg * HALF + (tt + 1) * TILE]
                    nc.tensor.matmul(h_ps[:], lhsT=w1a_sb[:], rhs=src,
                                     start=True, stop=False)
                    stc = st_sb[:, wg * WIN + t * TILE : wg * WIN + (t + 1) * TILE]
                    nc.tensor.matmul(h_ps[:], lhsT=q_sb[:], rhs=stc,
                                     start=False, stop=True)

                    # ---- gelu (+b1), cast to bf16
                    hp = wpool.tile([128, TILE], dt.bfloat16, tag="hp")
                    nc.scalar.activation(
                        hp[:], h_ps[:],
                        func=mybir.ActivationFunctionType.Gelu,
                        bias=0.0, scale=1.0,
                    )

                    if "nom2" in ABLATE:
                        continue
                    # ---- M2: y = h'.T @ W2 (pivot: edges onto partitions)
                    y_ps = ypsum.tile([128, n_ch, O], dt.float32, tag="y")
                    for c in range(n_ch):
                        nc.tensor.matmul(
                            y_ps[:, c, :],
                            lhsT=hp[:, c * CH : (c + 1) * CH], rhs=w2_sb[:],
                            start=True, stop=True,
                        )

                    gc0 = wi * (WIN // 128) + t * n_ch
                    ysc = wpool.tile([128, n_ch, O], dt.bfloat16, tag="ysc")
                    nc.vector.tensor_tensor(
                        out=ysc[:], in0=y_ps[:],
                        in1=w_sb[:, gc0 : gc0 + n_ch].to_broadcast([128, n_ch, O]),
                        op=mybir.AluOpType.mult,
                    )

                    if "nom3" in ABLATE:
                        continue
                    # ---- M3 segment accumulate (S streamed from host)
                    sm0 = (wg * WIN + t * TILE) // 128 * 128
                    for c in range(n_ch):
                        nc.tensor.matmul(
                            win_ps[:],
                            lhsT=sme_sb[:, sm0 + c * CH : sm0 + (c + 1) * CH],
                            rhs=ysc[:, c, :],
                            start=(t == 0 and c == 0),
                            stop=(t == n_tiles - 1 and c == n_ch - 1),
                            skip_group_check=True,
                        )

                # ---- flush window into the group staging tile
                if ABLATE & {"nom2", "nom3"}:
                    continue
                nc.scalar.activation(flst[:, wg, :], win_ps[:],
                                     func=mybir.ActivationFunctionType.Copy)
            if not (ABLATE & {"nom2", "nom3"}):
                nc.sync.dma_start(
                    out=d_out[g * GRP * 128 : (g + 1) * GRP * 128, :]
                        .rearrange("(w p) o -> p w o", p=128),
                    in_=flst[:],
                )

    nc.compile()
    return nc


# ------------------------------------------------------------------- runner

LAST_RESULT = None


def kernel(in_features, out_features, W1, b1, W2, b2,
           neighbors_index, neighbors_row_splits):
    import os
    from concourse.bass_utils import run_bass_kernel_spmd

    in_maps, metas, nwin, s_tab, counts = _host_prep(
        in_features, out_features, W1, b1, W2, b2,
        neighbors_index, neighbors_row_splits,
    )

    key = (nwin, s_tab)
    if key not in _prog_cache:
        _prog_cache[key] = _build_program(nwin, s_tab)
    nc = _prog_cache[key]

    trace = bool(os.environ.get("KERNEL_TRACE"))
    if trace:
        try:
            import antenv.axon_hooks  # noqa: F401
        except ImportError:
            trace = False
    res = run_bass_kernel_spmd(nc, in_maps, core_ids=list(range(NCORES)),
                               trace=trace)
    global LAST_RESULT
    LAST_RESULT = res
    outs = res.results

    out = np.zeros((M, O), np.float32)
    bounds = [round(k * E / NCORES) for k in range(NCORES + 1)]
    for k in range(NCORES):
        b = metas[k]
        slots = np.asarray(outs[k]["out_slots"], np.float32)
        for wi in range(b["n_real"]):
            base = int(b["bases"][wi])
            span = int(b["spans"][wi])
            out[base : base + span] += slots[wi * 128 : wi * 128 + span]

    b2v = np.asarray(b2, np.float32)
    out += b2v[None, :] * (counts > 0)[:, None].astype(np.float32)
    return out



# revision 4
# speedup vs baseline: 2.0510x; 2.0510x over previous
"""NeighborMLPConvLayer Trainium2 kernel (v2).

Strategy (8 NeuronCores, SPMD, edge-parallel):
  - Edges (sorted by destination segment) are split into 8 equal contiguous
    ranges; boundary segments are fixed up by a host-side overlap-add.
  - Per core, edges are packed into 128-edge "chunks"; a chunk never spans
    more than 16 distinct segments (cut + pad otherwise, which is rare at
    avg degree 32).  16 chunks form a 2048-slot window.
  - The HOST gathers features per edge into a dense bf16 stream
    comb[65, slots]: rows 0-31 in_features[idx], rows 32-63
    out_features[seg], row 64 ones (injects b1 via W1cat row 64).  Dense
    streams run at full DMA bandwidth (no per-row gather descriptors).
  - Device per chunk: h = comb_chunk.T @ W1cat (K=65) -> PSUM [128e, H];
    gelu on Act -> SBUF bf16.
  - Segment-sum BEFORE W2 (linearity): hsT[H, 16] = h_chunk.T @ onehot16
    where onehot16 is a per-chunk [128e, 16s] fp8 one-hot (16 B/edge).
    Per window the 16 chunk slabs land in one PSUM tile [H, 256].
  - M2: y[128, O] = hsT_sb.T @ W2 twice per window (256 chunk-seg rows).
  - Host: overlap-add chunk slabs (base + 16 rows each) into out[M, O]
    via per-column bincount, then divide by counts and add b2.
"""

import sys

sys.path.insert(0, "/opt/trn_rl_repo")

import numpy as np
import ml_dtypes

BF16 = ml_dtypes.bfloat16
FP8 = ml_dtypes.float8_e4m3

# Problem geometry (hardcoded per the task contract).
N = 50000
M = 50000
C = 32
H = 128
O = 64
E = 1_600_000
NCORES = 8

CHUNK = 128            # edges per chunk (PE partition dim)
SEGW = 16              # max segments spanned by one chunk (one-hot width)
CPW = 16               # chunks per window
WIN = CHUNK * CPW      # 2048 edge slots per window
GRP = 2                # windows per DMA group
KC = C + C + 1         # comb rows: rep(32) + slf(32) + ones(1)

_prog_cache = {}


# ----------------------------------------------------------------- host prep

def _cut_chunks(seg_c):
    """Greedy 128-edge chunks, each spanning < SEGW segments.

    Returns (starts, ends, bases) arrays.
    """
    n = seg_c.shape[0]
    starts, ends, bases = [], [], []
    p = 0
    while p < n:
        b = int(seg_c[p])
        q = int(np.searchsorted(seg_c, b + SEGW, side="left"))
        cut = min(p + CHUNK, q, n)
        assert cut > p
        starts.append(p)
        ends.append(cut)
        bases.append(b)
        p = cut
    return (np.asarray(starts, np.int64), np.asarray(ends, np.int64),
            np.asarray(bases, np.int64))


def _host_prep(in_features, out_features, W1, b1, W2, b2,
               neighbors_index, neighbors_row_splits):
    rs = np.asarray(neighbors_row_splits).astype(np.int64)
    idx_all = np.asarray(neighbors_index).astype(np.int64)
    counts = np.diff(rs)
    seg_all = np.repeat(np.arange(M, dtype=np.int64), counts)

    bounds = [round(k * E / NCORES) for k in range(NCORES + 1)]

    cores = []
    nwin = 0
    for k in range(NCORES):
        lo, hi = bounds[k], bounds[k + 1]
        seg_c = seg_all[lo:hi]
        starts, ends, bases = _cut_chunks(seg_c)
        nch = starts.shape[0]
        nwin = max(nwin, -(-nch // CPW))
        cores.append((idx_all[lo:hi], seg_c, starts, ends, bases))
    nwin = -(-nwin // GRP) * GRP

    inF = np.asarray(in_features, np.float32).astype(BF16)
    outF = np.asarray(out_features, np.float32).astype(BF16)

    w1 = np.asarray(W1, np.float32)
    w1cat = np.concatenate([w1, np.asarray(b1, np.float32).reshape(1, H)], 0)
    consts = dict(
        w1cat=np.ascontiguousarray(w1cat).astype(BF16),
        w2=np.asarray(W2, np.float32).astype(BF16),
    )

    in_maps = []
    metas = []
    for k in range(NCORES):
        idx_c, seg_c, starts, ends, bases = cores[k]
        n = idx_c.shape[0]
        nch = starts.shape[0]
        ncs = ends - starts
        chunk_ids = np.repeat(np.arange(nch, dtype=np.int64), ncs)
        within = np.arange(n, dtype=np.int64) - np.repeat(starts, ncs)
        slots = chunk_ids * CHUNK + within

        comb = np.zeros((KC, nwin * WIN), BF16)
        comb[0:C, slots] = inF[idx_c].T
        comb[C:2 * C, slots] = outF[seg_c].T
        comb[2 * C, slots] = np.float32(1.0)

        sloc = seg_c - bases[chunk_ids]
        assert int(sloc.max()) < SEGW
        sme = np.zeros((CHUNK, nwin * CPW * SEGW), FP8)
        sme[within, chunk_ids * SEGW + sloc] = np.float32(1.0)

        bases_full = np.zeros(nwin * CPW, np.int64)
        bases_full[:nch] = bases

        in_maps.append(dict(comb=comb, sme=sme, **consts))
        metas.append(dict(bases=bases_full, nch=nch))

    return in_maps, metas, nwin, counts


# ------------------------------------------------------------ device program

def _build_program(nwin):
    import concourse.bacc as bacc
    import concourse.mybir as mybir
    import concourse.tile as tile

    dt = mybir.dt
    nc = bacc.Bacc("TRN2", target_bir_lowering=False, debug=False)

    d_comb = nc.dram_tensor("comb", [KC, nwin * WIN], dt.bfloat16,
                            kind="ExternalInput")
    d_sme = nc.dram_tensor("sme", [CHUNK, nwin * CPW * SEGW], dt.float8e4,
                           kind="ExternalInput")
    d_w1cat = nc.dram_tensor("w1cat", [KC, H], dt.bfloat16,
                             kind="ExternalInput")
    d_w2 = nc.dram_tensor("w2", [H, O], dt.bfloat16, kind="ExternalInput")
    d_y = nc.dram_tensor("yout", [CHUNK, nwin * 2 * O], dt.float32,
                         kind="ExternalOutput")

    from contextlib import ExitStack

    HB = 8  # chunks per gelu batch (half window)

    with tile.TileContext(nc) as tc, ExitStack() as ctx:
        cpool = ctx.enter_context(tc.tile_pool(name="consts", bufs=1))
        gpool = ctx.enter_context(tc.tile_pool(name="stream", bufs=3))
        hpool = ctx.enter_context(tc.tile_pool(name="hsb", bufs=3))
        spool = ctx.enter_context(tc.tile_pool(name="small", bufs=3))
        ypool = ctx.enter_context(tc.tile_pool(name="ystage", bufs=3))
        hpsum = ctx.enter_context(tc.tile_pool(name="hpsum", bufs=2,
                                               space="PSUM"))
        spsum = ctx.enter_context(tc.tile_pool(name="spsum", bufs=2,
                                               space="PSUM"))
        ypsum = ctx.enter_context(tc.tile_pool(name="ypsum", bufs=2,
                                               space="PSUM"))

        w1_sb = cpool.tile([KC, H], dt.bfloat16, tag="w1")
        w2_sb = cpool.tile([H, O], dt.bfloat16, tag="w2")
        nc.sync.dma_start(out=w1_sb[:], in_=d_w1cat[:])
        nc.sync.dma_start(out=w2_sb[:], in_=d_w2[:])

        for g in range(nwin // GRP):
            comb_t = gpool.tile([KC, GRP * WIN], dt.bfloat16, tag="comb")
            nc.sync.dma_start(
                out=comb_t[:],
                in_=d_comb[:, g * GRP * WIN:(g + 1) * GRP * WIN])
            sme_t = gpool.tile([CHUNK, GRP * CPW * SEGW], dt.float8e4,
                               tag="sme")
            nc.gpsimd.dma_start(
                out=sme_t[:],
                in_=d_sme[:, g * GRP * CPW * SEGW:(g + 1) * GRP * CPW * SEGW])

            y_sb = ypool.tile([CHUNK, GRP * 2, O], dt.float32, tag="ysb")
            for w in range(GRP):
                hsT_ps = spsum.tile([H, CPW, SEGW], dt.float32, tag="hsT")
                for half in range(2):
                    h_ps = hpsum.tile([CHUNK, HB, H], dt.float32, tag="h")
                    for c8 in range(HB):
                        c = half * HB + c8
                        e0 = w * WIN + c * CHUNK
                        nc.tensor.matmul(
                            h_ps[:, c8, :],
                            lhsT=comb_t[:, e0:e0 + CHUNK],
                            rhs=w1_sb[:],
                            start=True, stop=True,
                        )
                    h_sb = hpool.tile([CHUNK, HB, H], dt.bfloat16, tag="hsb")
                    nc.scalar.activation(
                        h_sb[:], h_ps[:],
                        func=mybir.ActivationFunctionType.Gelu,
                        bias=0.0, scale=1.0,
                    )
                    for c8 in range(HB):
                        c = half * HB + c8
                        s0 = w * CPW * SEGW + c * SEGW
                        nc.tensor.matmul(
                            hsT_ps[:, c, :],
                            lhsT=h_sb[:, c8, :],
                            rhs=sme_t[:, s0:s0 + SEGW],
                            start=True, stop=True,
                        )
                hsT_sb = spool.tile([H, CPW, SEGW], dt.bfloat16, tag="hsTsb")
                nc.vector.tensor_copy(out=hsT_sb[:], in_=hsT_ps[:])
                y_ps = ypsum.tile([CHUNK, 2, O], dt.float32, tag="y")
                for s2 in range(2):
                    nc.tensor.matmul(
                        y_ps[:, s2, :],
                        lhsT=hsT_sb[:, s2 * 8:(s2 + 1) * 8, :],
                        rhs=w2_sb[:],
                        start=True, stop=True,
                    )
                nc.vector.tensor_copy(out=y_sb[:, w * 2:(w + 1) * 2, :],
                                      in_=y_ps[:])
            nc.scalar.dma_start(
                out=d_y[:, g * GRP * 2 * O:(g + 1) * GRP * 2 * O],
                in_=y_sb[:])

    nc.compile()
    return nc


# ------------------------------------------------------------------- runner

LAST_RESULT = None


def kernel(in_features, out_features, W1, b1, W2, b2,
           neighbors_index, neighbors_row_splits):
    import os
    from concourse.bass_utils import run_bass_kernel_spmd

    in_maps, metas, nwin, counts = _host_prep(
        in_features, out_features, W1, b1, W2, b2,
        neighbors_index, neighbors_row_splits,
    )

    if nwin not in _prog_cache:
        _prog_cache[nwin] = _build_program(nwin)
    nc = _prog_cache[nwin]

    trace = bool(os.environ.get("KERNEL_TRACE"))
    if trace:
        try:
            import antenv.axon_hooks  # noqa: F401
        except ImportError:
            trace = False
    res = run_bass_kernel_spmd(nc, in_maps, core_ids=list(range(NCORES)),
                               trace=trace)
    global LAST_RESULT
    LAST_RESULT = res
    outs = res.results

    acc = np.zeros((M, O), np.float64)
    for k in range(NCORES):
        # yout [128, nwin*2, O] -> rows r = s2*128 + p of window w map to
        # chunk = s2*8 + p//16, seg = bases[w*CPW + chunk] + p % 16.
        y = np.asarray(outs[k]["yout"], np.float32)
        y = y.reshape(CHUNK, nwin, 2, O).transpose(1, 2, 0, 3)
        y = np.ascontiguousarray(y).reshape(nwin * 2 * CHUNK, O)
        p = np.tile(np.arange(CHUNK), nwin * 2)
        s2 = np.tile(np.repeat(np.arange(2), CHUNK), nwin)
        w = np.repeat(np.arange(nwin), 2 * CHUNK)
        chunk = w * CPW + s2 * 8 + p // SEGW
        gidx = metas[k]["bases"][chunk] + p % SEGW
        for o in range(O):
            # rows past a chunk's actual span are all-zero; indices may run
            # past M-1 for chunks near the end — truncate.
            acc[:, o] += np.bincount(gidx, weights=y[:, o].astype(np.float64),
                                     minlength=M)[:M]

    denom = np.maximum(counts, 1).astype(np.float64)
    out = (acc / denom[:, None]).astype(np.float32)
    b2v = np.asarray(b2, np.float32)
    out += b2v[None, :] * (counts > 0)[:, None].astype(np.float32)
    return out


# revision 12
# speedup vs baseline: 2.3406x; 1.1412x over previous
"""NeighborMLPConvLayer Trainium2 kernel (v3).

Strategy (8 NeuronCores, SPMD, edge-parallel):
  - Edges (sorted by destination segment) are split into 8 equal contiguous
    ranges; boundary segments are fixed up by a host-side overlap-add.
  - Per core, edges are packed into 128-edge "chunks"; a chunk never spans
    more than 16 distinct segments (cut + pad otherwise, which is rare at
    avg degree 32).  12 chunks form a 1536-slot window.
  - The HOST gathers features per edge into a dense bf16 stream
    comb[65, slots]: rows 0-31 in_features[idx], rows 32-63
    out_features[seg], row 64 ones (injects b1 via W1cat row 64).  Dense
    streams run at full DMA bandwidth (no per-row gather descriptors).
  - Device per chunk: h = comb_chunk.T @ W1cat (K=65) -> PSUM [128e, H];
    one gelu per window on Act (the bottleneck engine) -> SBUF bf16.
  - Segment-sum BEFORE W2 (linearity): hsT[H, 16] = h_chunk.T @ onehot16
    where onehot16 is a per-chunk [128e, 16s] fp8 one-hot (16 B/edge).
    Per window the 12 chunk slabs land in one PSUM scratch tile.
  - M2: y[96, O] = hsT_sb.T @ W2 twice per window (192 chunk-seg rows).
  - The loop is software-pipelined with a 1-window skew (PE is in-order:
    segsum(k-1), which waits on gelu(k-1), is emitted after M1(k) so the
    gelu hides behind the next window's M1).
  - Host: overlap-add chunk slabs (base + 16 rows each) into out[M, O]
    via per-column bincount, then divide by counts and add b2.
"""

import sys

sys.path.insert(0, "/opt/trn_rl_repo")

import numpy as np
import ml_dtypes

BF16 = ml_dtypes.bfloat16
FP8 = ml_dtypes.float8_e4m3

# Problem geometry (hardcoded per the task contract).
N = 50000
M = 50000
C = 32
H = 128
O = 64
E = 1_600_000
NCORES = 8

CHUNK = 128            # edges per chunk (PE partition dim)
SEGW = 16              # max segments spanned by one chunk (one-hot width)
CPW = 12               # chunks per window (PSUM: 12*128*4B = 3 banks)
WIN = CHUNK * CPW      # 1536 edge slots per window
GRP = 2                # windows per DMA group
KC = C + C + 1         # comb rows: rep(32) + slf(32) + ones(1)
M2R = CPW * SEGW // 2  # 96 output rows per M2 matmul

_prog_cache = {}


# ----------------------------------------------------------------- host prep

def _cut_chunks(seg_c):
    """Greedy 128-edge chunks, each spanning < SEGW segments.

    Returns (starts, ends, bases) arrays.
    """
    n = seg_c.shape[0]
    starts, ends, bases = [], [], []
    p = 0
    while p < n:
        b = int(seg_c[p])
        q = int(np.searchsorted(seg_c, b + SEGW, side="left"))
        cut = min(p + CHUNK, q, n)
        assert cut > p
        starts.append(p)
        ends.append(cut)
        bases.append(b)
        p = cut
    return (np.asarray(starts, np.int64), np.asarray(ends, np.int64),
            np.asarray(bases, np.int64))


def _host_prep(in_features, out_features, W1, b1, W2, b2,
               neighbors_index, neighbors_row_splits):
    rs = np.asarray(neighbors_row_splits).astype(np.int64)
    idx_all = np.asarray(neighbors_index).astype(np.int64)
    counts = np.diff(rs)
    seg_all = np.repeat(np.arange(M, dtype=np.int64), counts)

    bounds = [round(k * E / NCORES) for k in range(NCORES + 1)]

    cores = []
    nwin = 0
    for k in range(NCORES):
        lo, hi = bounds[k], bounds[k + 1]
        seg_c = seg_all[lo:hi]
        starts, ends, bases = _cut_chunks(seg_c)
        nch = starts.shape[0]
        nwin = max(nwin, -(-nch // CPW))
        cores.append((idx_all[lo:hi], seg_c, starts, ends, bases))
    nwin = -(-nwin // GRP) * GRP

    inF = np.asarray(in_features, np.float32).astype(BF16)
    outF = np.asarray(out_features, np.float32).astype(BF16)

    w1 = np.asarray(W1, np.float32)
    w1cat = np.concatenate([w1, np.asarray(b1, np.float32).reshape(1, H)], 0)
    consts = dict(
        w1cat=np.ascontiguousarray(w1cat).astype(BF16),
        w2=np.asarray(W2, np.float32).astype(BF16),
    )

    in_maps = []
    metas = []
    for k in range(NCORES):
        idx_c, seg_c, starts, ends, bases = cores[k]
        n = idx_c.shape[0]
        nch = starts.shape[0]
        ncs = ends - starts
        chunk_ids = np.repeat(np.arange(nch, dtype=np.int64), ncs)
        within = np.arange(n, dtype=np.int64) - np.repeat(starts, ncs)
        slots = chunk_ids * CHUNK + within

        comb = np.zeros((KC, nwin * WIN), BF16)
        comb[0:C, slots] = inF[idx_c].T
        comb[C:2 * C, slots] = outF[seg_c].T
        comb[2 * C, slots] = np.float32(1.0)

        sloc = seg_c - bases[chunk_ids]
        assert int(sloc.max()) < SEGW
        sme = np.zeros((CHUNK, nwin * CPW * SEGW), FP8)
        sme[within, chunk_ids * SEGW + sloc] = np.float32(1.0)

        bases_full = np.zeros(nwin * CPW, np.int64)
        bases_full[:nch] = bases

        in_maps.append(dict(comb=comb, sme=sme, **consts))
        metas.append(dict(bases=bases_full, nch=nch))

    return in_maps, metas, nwin, counts


# ------------------------------------------------------------ device program

def _build_program(nwin):
    import concourse.bacc as bacc
    import concourse.mybir as mybir
    import concourse.tile as tile

    dt = mybir.dt
    nc = bacc.Bacc("TRN2", target_bir_lowering=False, debug=False)

    d_comb = nc.dram_tensor("comb", [KC, nwin * WIN], dt.bfloat16,
                            kind="ExternalInput")
    d_sme = nc.dram_tensor("sme", [CHUNK, nwin * CPW * SEGW], dt.float8e4,
                           kind="ExternalInput")
    d_w1cat = nc.dram_tensor("w1cat", [KC, H], dt.bfloat16,
                             kind="ExternalInput")
    d_w2 = nc.dram_tensor("w2", [H, O], dt.bfloat16, kind="ExternalInput")
    d_y = nc.dram_tensor("yout", [M2R, nwin * 2 * O], dt.float32,
                         kind="ExternalOutput")

    from contextlib import ExitStack

    with tile.TileContext(nc) as tc, ExitStack() as ctx:
        cpool = ctx.enter_context(tc.tile_pool(name="consts", bufs=1))
        gpool = ctx.enter_context(tc.tile_pool(name="stream", bufs=3))
        hpool = ctx.enter_context(tc.tile_pool(name="hsb", bufs=3))
        spool = ctx.enter_context(tc.tile_pool(name="small", bufs=3))
        ypool = ctx.enter_context(tc.tile_pool(name="ystage", bufs=3))
        hpsum = ctx.enter_context(tc.tile_pool(name="hpsum", bufs=2,
                                               space="PSUM"))
        wpsum = ctx.enter_context(tc.tile_pool(name="wpsum", bufs=2,
                                               space="PSUM"))

        w1_sb = cpool.tile([KC, H], dt.bfloat16, tag="w1")
        w2_sb = cpool.tile([H, O], dt.bfloat16, tag="w2")
        nc.sync.dma_start(out=w1_sb[:], in_=d_w1cat[:])
        nc.sync.dma_start(out=w2_sb[:], in_=d_w2[:])

        # Warm the Gelu activation table while the first streams are in
        # flight (table load is ~1.3us and otherwise serializes before the
        # first real gelu).
        warm = cpool.tile([1, 2], dt.bfloat16, tag="warm")
        nc.gpsimd.memset(warm[:], 0.0)
        nc.scalar.activation(warm[:], warm[:],
                             func=mybir.ActivationFunctionType.Gelu,
                             bias=0.0, scale=1.0)

        tiles = {}   # group -> (comb_t, sme_t)
        ysbs = {}    # group -> y_sb staging tile
        pend = None  # (k, h_ps, h_sb) waiting for its segsum/M2 phase

        def fetch_group(g):
            comb_t = gpool.tile([KC, GRP * WIN], dt.bfloat16, tag="comb")
            nc.sync.dma_start(
                out=comb_t[:],
                in_=d_comb[:, g * GRP * WIN:(g + 1) * GRP * WIN])
            sme_t = gpool.tile([CHUNK, GRP * CPW * SEGW], dt.float8e4,
                               tag="sme")
            nc.gpsimd.dma_start(
                out=sme_t[:],
                in_=d_sme[:, g * GRP * CPW * SEGW:
                          (g + 1) * GRP * CPW * SEGW])
            tiles[g] = (comb_t, sme_t)

        def finish_window(k, h_ps, h_sb):
            g = k // GRP
            w = k % GRP
            sme_t = tiles[g][1]
            # combined scratch: hsT [128, CPW*16] f32 + y [96, 2, 64] f32 in
            # one PSUM bank.
            scr = wpsum.tile([CHUNK, CPW * SEGW + 2 * O], dt.float32,
                             tag="scr", name=f"scr{k}")
            hsT_ps = scr[:, 0:CPW * SEGW]
            for c in range(CPW):
                nc.tensor.matmul(
                    hsT_ps[:, c * SEGW:(c + 1) * SEGW],
                    lhsT=h_sb[:, c, :],
                    rhs=sme_t[:, (w * CPW + c) * SEGW:
                              (w * CPW + c + 1) * SEGW],
                    start=True, stop=True,
                    skip_group_check=True,
                )
            hsT_sb = spool.tile([H, CPW, SEGW], dt.bfloat16, tag="hsTsb")
            nc.vector.tensor_copy(out=hsT_sb[:],
                                  in_=hsT_ps.rearrange(
                                      "p (a b) -> p a b", a=CPW))
            y_ps = scr[0:M2R, CPW * SEGW:].rearrange("p (a b) -> p a b", a=2)
            for s2 in range(2):
                nc.tensor.matmul(
                    y_ps[:, s2, :],
                    lhsT=hsT_sb[:, s2 * (CPW // 2):(s2 + 1) * (CPW // 2), :],
                    rhs=w2_sb[:],
                    start=True, stop=True,
                    skip_group_check=True,
                )
            if g not in ysbs:
                ysbs[g] = ypool.tile([M2R, GRP * 2, O], dt.float32,
                                     tag="ysb", name=f"ysb{g}")
            nc.vector.tensor_copy(out=ysbs[g][:, w * 2:(w + 1) * 2, :],
                                  in_=y_ps)
            if w == GRP - 1:
                nc.sync.dma_start(
                    out=d_y[:, g * GRP * 2 * O:(g + 1) * GRP * 2 * O],
                    in_=ysbs.pop(g)[:])
                tiles.pop(g)

        for k in range(nwin):
            g, w = k // GRP, k % GRP
            if w == 0:
                fetch_group(g)
            comb_t = tiles[g][0]
            h_ps = hpsum.tile([CHUNK, CPW, H], dt.float32, tag="h")
            for c in range(CPW):
                e0 = w * WIN + c * CHUNK
                nc.tensor.matmul(
                    h_ps[:, c, :],
                    lhsT=comb_t[:, e0:e0 + CHUNK],
                    rhs=w1_sb[:],
                    start=True, stop=True,
                )
            h_sb = hpool.tile([CHUNK, CPW, H], dt.bfloat16, tag="hsb")
            nc.scalar.activation(
                h_sb[:], h_ps[:],
                func=mybir.ActivationFunctionType.Gelu,
                bias=0.0, scale=1.0,
            )
            if pend is not None:
                finish_window(*pend)
            pend = (k, h_ps, h_sb)
        finish_window(*pend)

    nc.compile()
    return nc


# ------------------------------------------------------------------- runner

LAST_RESULT = None


def kernel(in_features, out_features, W1, b1, W2, b2,
           neighbors_index, neighbors_row_splits):
    import os
    from concourse.bass_utils import run_bass_kernel_spmd

    in_maps, metas, nwin, counts = _host_prep(
        in_features, out_features, W1, b1, W2, b2,
        neighbors_index, neighbors_row_splits,
    )

    if nwin not in _prog_cache:
        _prog_cache[nwin] = _build_program(nwin)
    nc = _prog_cache[nwin]

    trace = bool(os.environ.get("KERNEL_TRACE"))
    if trace:
        try:
            import antenv.axon_hooks  # noqa: F401
        except ImportError:
            trace = False
    res = run_bass_kernel_spmd(nc, in_maps, core_ids=list(range(NCORES)),
                               trace=trace)
    global LAST_RESULT
    LAST_RESULT = res
    outs = res.results

    acc = np.zeros((M, O), np.float64)
    for k in range(NCORES):
        # yout [96, nwin*2, O] -> rows r = (w, s2, p) map to
        # chunk = w*CPW + s2*6 + p//16, seg = bases[chunk] + p % 16.
        y = np.asarray(outs[k]["yout"], np.float32)
        y = y.reshape(M2R, nwin, 2, O).transpose(1, 2, 0, 3)
        y = np.ascontiguousarray(y).reshape(nwin * 2 * M2R, O)
        p = np.tile(np.arange(M2R), nwin * 2)
        s2 = np.tile(np.repeat(np.arange(2), M2R), nwin)
        w = np.repeat(np.arange(nwin), 2 * M2R)
        chunk = w * CPW + s2 * (CPW // 2) + p // SEGW
        gidx = metas[k]["bases"][chunk] + p % SEGW
        for o in range(O):
            # rows past a chunk's actual span are all-zero; indices may run
            # past M-1 for chunks near the end — truncate.
            acc[:, o] += np.bincount(gidx, weights=y[:, o].astype(np.float64),
                                     minlength=M)[:M]

    denom = np.maximum(counts, 1).astype(np.float64)
    out = (acc / denom[:, None]).astype(np.float32)
    b2v = np.asarray(b2, np.float32)
    out += b2v[None, :] * (counts > 0)[:, None].astype(np.float32)
    return out


# revision 17
# speedup vs baseline: 2.3933x; 1.0225x over previous
"""NeighborMLPConvLayer Trainium2 kernel (v3).

Strategy (8 NeuronCores, SPMD, edge-parallel):
  - Edges (sorted by destination segment) are split into 8 equal contiguous
    ranges; boundary segments are fixed up by a host-side overlap-add.
  - Per core, edges are packed into 128-edge "chunks"; a chunk never spans
    more than 16 distinct segments (cut + pad otherwise, which is rare at
    avg degree 32).  12 chunks form a 1536-slot window.
  - The HOST gathers features per edge into a dense bf16 stream
    comb[65, slots]: rows 0-31 in_features[idx], rows 32-63
    out_features[seg], row 64 ones (injects b1 via W1cat row 64).  Dense
    streams run at full DMA bandwidth (no per-row gather descriptors).
  - Device per chunk: h = comb_chunk.T @ W1cat (K=65) -> PSUM [128e, H];
    one gelu per window on Act (the bottleneck engine) -> SBUF bf16.
  - Segment-sum BEFORE W2 (linearity): hsT[H, 16] = h_chunk.T @ onehot16
    where onehot16 is a per-chunk [128e, 16s] fp8 one-hot (16 B/edge).
    Per window the 12 chunk slabs land in one PSUM scratch tile.
  - M2: y[96, O] = hsT_sb.T @ W2 twice per window (192 chunk-seg rows).
  - The loop is software-pipelined with a 1-window skew (PE is in-order:
    segsum(k-1), which waits on gelu(k-1), is emitted after M1(k) so the
    gelu hides behind the next window's M1).
  - Host: overlap-add chunk slabs (base + 16 rows each) into out[M, O]
    via per-column bincount, then divide by counts and add b2.
"""

import sys

sys.path.insert(0, "/opt/trn_rl_repo")

import numpy as np
import ml_dtypes

BF16 = ml_dtypes.bfloat16
FP8 = ml_dtypes.float8_e4m3

# Problem geometry (hardcoded per the task contract).
N = 50000
M = 50000
C = 32
H = 128
O = 64
E = 1_600_000
NCORES = 8

CHUNK = 128            # edges per chunk (PE partition dim)
SEGW = 16              # max segments spanned by one chunk (one-hot width)
CPW = 12               # chunks per window (PSUM: 12*128*4B = 3 banks)
WIN = CHUNK * CPW      # 1536 edge slots per window
GRP = 2                # windows per DMA group
KC = C + C + 1         # comb rows: rep(32) + slf(32) + ones(1)
M2R = CPW * SEGW // 2  # 96 output rows per M2 matmul

_prog_cache = {}


# ----------------------------------------------------------------- host prep

def _cut_chunks(seg_c):
    """Greedy 128-edge chunks, each spanning < SEGW segments.

    Returns (starts, ends, bases) arrays.
    """
    n = seg_c.shape[0]
    starts, ends, bases = [], [], []
    p = 0
    while p < n:
        b = int(seg_c[p])
        q = int(np.searchsorted(seg_c, b + SEGW, side="left"))
        cut = min(p + CHUNK, q, n)
        assert cut > p
        starts.append(p)
        ends.append(cut)
        bases.append(b)
        p = cut
    return (np.asarray(starts, np.int64), np.asarray(ends, np.int64),
            np.asarray(bases, np.int64))


def _host_prep(in_features, out_features, W1, b1, W2, b2,
               neighbors_index, neighbors_row_splits):
    rs = np.asarray(neighbors_row_splits).astype(np.int64)
    idx_all = np.asarray(neighbors_index).astype(np.int64)
    counts = np.diff(rs)
    seg_all = np.repeat(np.arange(M, dtype=np.int64), counts)

    bounds = [round(k * E / NCORES) for k in range(NCORES + 1)]

    cores = []
    nwin = 0
    for k in range(NCORES):
        lo, hi = bounds[k], bounds[k + 1]
        seg_c = seg_all[lo:hi]
        starts, ends, bases = _cut_chunks(seg_c)
        nch = starts.shape[0]
        nwin = max(nwin, -(-nch // CPW))
        cores.append((idx_all[lo:hi], seg_c, starts, ends, bases))

    inF = np.asarray(in_features, np.float32).astype(BF16)
    outF = np.asarray(out_features, np.float32).astype(BF16)

    w1 = np.asarray(W1, np.float32)
    w1cat = np.concatenate([w1, np.asarray(b1, np.float32).reshape(1, H)], 0)
    consts = dict(
        w1cat=np.ascontiguousarray(w1cat).astype(BF16),
        w2=np.asarray(W2, np.float32).astype(BF16),
    )

    in_maps = []
    metas = []
    for k in range(NCORES):
        idx_c, seg_c, starts, ends, bases = cores[k]
        n = idx_c.shape[0]
        nch = starts.shape[0]
        ncs = ends - starts
        chunk_ids = np.repeat(np.arange(nch, dtype=np.int64), ncs)
        within = np.arange(n, dtype=np.int64) - np.repeat(starts, ncs)
        slots = chunk_ids * CHUNK + within

        comb = np.zeros((KC, nwin * WIN), BF16)
        comb[0:C, slots] = inF[idx_c].T
        comb[C:2 * C, slots] = outF[seg_c].T
        comb[2 * C, slots] = np.float32(1.0)

        sloc = seg_c - bases[chunk_ids]
        assert int(sloc.max()) < SEGW
        sme = np.zeros((CHUNK, nwin * CPW * SEGW), FP8)
        sme[within, chunk_ids * SEGW + sloc] = np.float32(1.0)

        bases_full = np.zeros(nwin * CPW, np.int64)
        bases_full[:nch] = bases

        in_maps.append(dict(comb=comb, sme=sme, **consts))
        metas.append(dict(bases=bases_full, nch=nch))

    return in_maps, metas, nwin, counts


# ------------------------------------------------------------ device program

def _build_program(nwin):
    import concourse.bacc as bacc
    import concourse.mybir as mybir
    import concourse.tile as tile

    dt = mybir.dt
    nc = bacc.Bacc("TRN2", target_bir_lowering=False, debug=False)

    d_comb = nc.dram_tensor("comb", [KC, nwin * WIN], dt.bfloat16,
                            kind="ExternalInput")
    d_sme = nc.dram_tensor("sme", [CHUNK, nwin * CPW * SEGW], dt.float8e4,
                           kind="ExternalInput")
    d_w1cat = nc.dram_tensor("w1cat", [KC, H], dt.bfloat16,
                             kind="ExternalInput")
    d_w2 = nc.dram_tensor("w2", [H, O], dt.bfloat16, kind="ExternalInput")
    d_y = nc.dram_tensor("yout", [M2R, nwin * 2 * O], dt.float32,
                         kind="ExternalOutput")

    from contextlib import ExitStack

    with tile.TileContext(nc) as tc, ExitStack() as ctx:
        cpool = ctx.enter_context(tc.tile_pool(name="consts", bufs=1))
        gpool = ctx.enter_context(tc.tile_pool(name="stream", bufs=3))
        hpool = ctx.enter_context(tc.tile_pool(name="hsb", bufs=3))
        spool = ctx.enter_context(tc.tile_pool(name="small", bufs=3))
        ypool = ctx.enter_context(tc.tile_pool(name="ystage", bufs=3))
        hpsum = ctx.enter_context(tc.tile_pool(name="hpsum", bufs=2,
                                               space="PSUM"))
        wpsum = ctx.enter_context(tc.tile_pool(name="wpsum", bufs=2,
                                               space="PSUM"))

        w1_sb = cpool.tile([KC, H], dt.bfloat16, tag="w1")
        w2_sb = cpool.tile([H, O], dt.bfloat16, tag="w2")
        # consts issue on Act's (idle) sequencer so the first comb stream is
        # not queued behind them on SP.
        nc.scalar.dma_start(out=w1_sb[:], in_=d_w1cat[:])
        nc.scalar.dma_start(out=w2_sb[:], in_=d_w2[:])

        # Warm the Gelu activation table while the first streams are in
        # flight (table load is ~1.3us and otherwise serializes before the
        # first real gelu).
        warm = cpool.tile([1, 2], dt.bfloat16, tag="warm")
        nc.gpsimd.memset(warm[:], 0.0)
        nc.scalar.activation(warm[:], warm[:],
                             func=mybir.ActivationFunctionType.Gelu,
                             bias=0.0, scale=1.0)

        # group g covers windows [g*GRP, g*GRP+gsz); the last group may be
        # smaller than GRP when nwin is not a multiple of GRP.
        ngrp = -(-nwin // GRP)
        gsize = [min(GRP, nwin - g * GRP) for g in range(ngrp)]

        tiles = {}   # group -> (comb_t, sme_t)
        ysbs = {}    # group -> y_sb staging tile
        pend = None  # (k, h_ps, h_sb) waiting for its segsum/M2 phase

        def fetch_group(g):
            gsz = gsize[g]
            w0 = g * GRP
            comb_t = gpool.tile([KC, gsz * WIN], dt.bfloat16,
                                tag=f"comb{gsz}", name=f"comb{g}")
            if g == 0:
                # split per window so the first M1 starts after 1/GRP of the
                # stream has landed (range-tracked dependencies).
                for w in range(gsz):
                    nc.sync.dma_start(
                        out=comb_t[:, w * WIN:(w + 1) * WIN],
                        in_=d_comb[:, (w0 + w) * WIN:(w0 + w + 1) * WIN])
            else:
                nc.sync.dma_start(
                    out=comb_t[:],
                    in_=d_comb[:, w0 * WIN:(w0 + gsz) * WIN])
            sme_t = gpool.tile([CHUNK, gsz * CPW * SEGW], dt.float8e4,
                               tag=f"sme{gsz}", name=f"sme{g}")
            nc.gpsimd.dma_start(
                out=sme_t[:],
                in_=d_sme[:, w0 * CPW * SEGW:(w0 + gsz) * CPW * SEGW])
            tiles[g] = (comb_t, sme_t)

        def finish_window(k, h_ps, h_sb):
            g = k // GRP
            w = k % GRP
            gsz = gsize[g]
            sme_t = tiles[g][1]
            # combined scratch: hsT [128, CPW*16] f32 + y [96, 2, 64] f32 in
            # one PSUM bank.
            scr = wpsum.tile([CHUNK, CPW * SEGW + 2 * O], dt.float32,
                             tag="scr", name=f"scr{k}")
            hsT_ps = scr[:, 0:CPW * SEGW]
            for c in range(CPW):
                nc.tensor.matmul(
                    hsT_ps[:, c * SEGW:(c + 1) * SEGW],
                    lhsT=h_sb[:, c, :],
                    rhs=sme_t[:, (w * CPW + c) * SEGW:
                              (w * CPW + c + 1) * SEGW],
                    start=True, stop=True,
                    skip_group_check=True,
                )
            hsT_sb = spool.tile([H, CPW, SEGW], dt.bfloat16, tag="hsTsb")
            nc.vector.tensor_copy(out=hsT_sb[:],
                                  in_=hsT_ps.rearrange(
                                      "p (a b) -> p a b", a=CPW))
            y_ps = scr[0:M2R, CPW * SEGW:].rearrange("p (a b) -> p a b", a=2)
            for s2 in range(2):
                nc.tensor.matmul(
                    y_ps[:, s2, :],
                    lhsT=hsT_sb[:, s2 * (CPW // 2):(s2 + 1) * (CPW // 2), :],
                    rhs=w2_sb[:],
                    start=True, stop=True,
                    skip_group_check=True,
                )
            if g not in ysbs:
                ysbs[g] = ypool.tile([M2R, gsz * 2, O], dt.float32,
                                     tag=f"ysb{gsz}", name=f"ysb{g}")
            nc.vector.tensor_copy(out=ysbs[g][:, w * 2:(w + 1) * 2, :],
                                  in_=y_ps)
            if w == gsz - 1:
                y0 = g * GRP * 2 * O
                nc.sync.dma_start(
                    out=d_y[:, y0:y0 + gsz * 2 * O],
                    in_=ysbs.pop(g)[:])
                tiles.pop(g)

        for k in range(nwin):
            g, w = k // GRP, k % GRP
            if w == 0:
                fetch_group(g)
            comb_t = tiles[g][0]
            h_ps = hpsum.tile([CHUNK, CPW, H], dt.float32, tag="h")
            for c in range(CPW):
                e0 = w * WIN + c * CHUNK
                nc.tensor.matmul(
                    h_ps[:, c, :],
                    lhsT=comb_t[:, e0:e0 + CHUNK],
                    rhs=w1_sb[:],
                    start=True, stop=True,
                )
            h_sb = hpool.tile([CHUNK, CPW, H], dt.bfloat16, tag="hsb")
            nc.scalar.activation(
                h_sb[:], h_ps[:],
                func=mybir.ActivationFunctionType.Gelu,
                bias=0.0, scale=1.0,
            )
            if pend is not None:
                finish_window(*pend)
            pend = (k, h_ps, h_sb)
        finish_window(*pend)

    nc.compile()
    return nc


# ------------------------------------------------------------------- runner

LAST_RESULT = None


def kernel(in_features, out_features, W1, b1, W2, b2,
           neighbors_index, neighbors_row_splits):
    import os
    from concourse.bass_utils import run_bass_kernel_spmd

    in_maps, metas, nwin, counts = _host_prep(
        in_features, out_features, W1, b1, W2, b2,
        neighbors_index, neighbors_row_splits,
    )

    if nwin not in _prog_cache:
        _prog_cache[nwin] = _build_program(nwin)
    nc = _prog_cache[nwin]

    trace = bool(os.environ.get("KERNEL_TRACE"))
    if trace:
        try:
            import antenv.axon_hooks  # noqa: F401
        except ImportError:
            trace = False
    res = run_bass_kernel_spmd(nc, in_maps, core_ids=list(range(NCORES)),
                               trace=trace)
    global LAST_RESULT
    LAST_RESULT = res
    outs = res.results

    acc = np.zeros((M, O), np.float64)
    for k in range(NCORES):
        # yout [96, nwin*2, O] -> rows r = (w, s2, p) map to
        # chunk = w*CPW + s2*6 + p//16, seg = bases[chunk] + p % 16.
        y = np.asarray(outs[k]["yout"], np.float32)
        y = y.reshape(M2R, nwin, 2, O).transpose(1, 2, 0, 3)
        y = np.ascontiguousarray(y).reshape(nwin * 2 * M2R, O)
        p = np.tile(np.arange(M2R), nwin * 2)
        s2 = np.tile(np.repeat(np.arange(2), M2R), nwin)
        w = np.repeat(np.arange(nwin), 2 * M2R)
        chunk = w * CPW + s2 * (CPW // 2) + p // SEGW
        gidx = metas[k]["bases"][chunk] + p % SEGW
        for o in range(O):
            # rows past a chunk's actual span are all-zero; indices may run
            # past M-1 for chunks near the end — truncate.
            acc[:, o] += np.bincount(gidx, weights=y[:, o].astype(np.float64),
                                     minlength=M)[:M]

    denom = np.maximum(counts, 1).astype(np.float64)
    out = (acc / denom[:, None]).astype(np.float32)
    b2v = np.asarray(b2, np.float32)
    out += b2v[None, :] * (counts > 0)[:, None].astype(np.float32)
    return out
